# revision 1
# baseline (speedup 1.0000x reference)
"""AttnAggregator2 Trainium2 kernel.

Math (per node n, with X[n, s, :] = table rows of [self, neigh_0..neigh_24]):
    Q       = table[node] @ Wq^T + bq
    scores  = Q . K  where K = X @ Wk^T + bk
            = (Q @ Wk) . X + (Q . bk)          <- Q.bk is constant per node and
                                                  cancels in softmax: dropped.
    attn    = softmax(scores)
    mix     = attn-weighted sum of V = (sum_s attn_s X_s) @ Wv^T + bv
                                                  (sum attn = 1 absorbs bv)

So the S+1 per-neighbor K/V projections collapse into three small dense
matmuls per node tile plus one elementwise product pass (scores) and one
PE "diagonal matmul" accumulation (the attn-weighted feature sum).

Sharding: data-parallel over nodes, 8 cores, table + weights replicated.

Per-core layout (node tiles of 128 on SBUF partitions):
    gather   G[p, s, :]  = table[idx[p, s]]           (indirect DMA, fp32)
    Q^T      = Wq @ Xself^T        (PE; Xself^T via PE transpose)
    Q'       = Q @ Wk              (PE, row layout [n, d])
    prod     = G * broadcast_s(Q')                    (DVE, fp32)
    scores   = reduce_d(prod)                         (DVE, fp32)
    attn     = softmax over s                         (DVE + ACT)
    diag_s   = diag(attn[:, s])   (DVE: bf16 identity x broadcast attn)
    Xmix^T   = sum_s (G_s)^T @ diag_s                 (PE, bf16, PSUM accum)
    out^T    = Wv @ Xmix^T + bv                       (PE fp32)
Output is written transposed [128, n]; host transposes back.
"""

import os
import sys
from contextlib import ExitStack

import numpy as np

sys.path.insert(0, "/opt/trn_rl_repo")

import concourse.bass as bass
import concourse.mybir as mybir
import concourse.tile as tile
from concourse import bacc
from concourse.bass_utils import run_bass_kernel_spmd
from concourse.masks import make_identity

F32 = mybir.dt.float32
BF16 = mybir.dt.bfloat16
I32 = mybir.dt.int32

VOCAB = 100000
N_NODES = 50000
S = 25
S1 = S + 1  # self + sampled neighbors
D = 128
P = 128
N_CORES = 8
N_PER_CORE = N_NODES // N_CORES  # 6250
N_TILES = (N_PER_CORE + P - 1) // P  # 49
N_PAD = N_TILES * P  # 6272


def build_kernel(n_tiles: int = N_TILES, vocab: int = VOCAB):
    nc = bacc.Bacc(
        "TRN2",
        target_bir_lowering=False,
        debug=False,
        enable_asserts=False,
    )

    table = nc.dram_tensor("table", [vocab, D], F32, kind="ExternalInput").ap()
    idx = nc.dram_tensor("idx", [P, n_tiles * S1], I32, kind="ExternalInput").ap()
    wqT = nc.dram_tensor("wqT", [D, D], F32, kind="ExternalInput").ap()
    wk = nc.dram_tensor("wk", [D, D], F32, kind="ExternalInput").ap()
    wvT = nc.dram_tensor("wvT", [D, D], F32, kind="ExternalInput").ap()
    bq = nc.dram_tensor("bq", [D, 1], F32, kind="ExternalInput").ap()
    bv = nc.dram_tensor("bv", [D, 1], F32, kind="ExternalInput").ap()
    out = nc.dram_tensor("out", [D, n_tiles * P], F32, kind="ExternalOutput").ap()

    with tile.TileContext(nc) as tc, ExitStack() as ctx:
        const = ctx.enter_context(tc.tile_pool(name="const", bufs=1))
        idxp = ctx.enter_context(tc.tile_pool(name="idxp", bufs=3))
        gpool = ctx.enter_context(tc.tile_pool(name="gpool", bufs=3))
        gbfp = ctx.enter_context(tc.tile_pool(name="gbfp", bufs=2))
        prodp = ctx.enter_context(tc.tile_pool(name="prodp", bufs=2))
        diagp = ctx.enter_context(tc.tile_pool(name="diagp", bufs=2))
        small = ctx.enter_context(tc.tile_pool(name="small", bufs=4))
        outp = ctx.enter_context(tc.tile_pool(name="outp", bufs=3))
        psum = ctx.enter_context(tc.tile_pool(name="psum", bufs=1, space="PSUM"))
        psum_xm = ctx.enter_context(tc.tile_pool(name="psum_xm", bufs=2, space="PSUM"))

        ident = const.tile([P, P], F32)
        make_identity(nc, ident[:])
        ident_bf = const.tile([P, P], BF16)
        nc.scalar.copy(ident_bf[:], ident[:])
        wqT_s = const.tile([D, D], F32)
        nc.sync.dma_start(wqT_s[:], wqT)
        wk_s = const.tile([D, D], F32)
        nc.sync.dma_start(wk_s[:], wk)
        wvT_s = const.tile([D, D], F32)
        nc.sync.dma_start(wvT_s[:], wvT)
        bq_s = const.tile([D, 1], F32)
        nc.sync.dma_start(bq_s[:], bq)
        bv_s = const.tile([D, 1], F32)
        nc.sync.dma_start(bv_s[:], bv)
        idx_all = const.tile([P, n_tiles * S1], I32)
        nc.sync.dma_start(idx_all[:], idx)

        for t in range(n_tiles):
            # Gather all S1 rows for 128 nodes: G[p, s, :] = table[idx[p, s]]
            # (one indirect DMA per s-slot: HW only supports one offset per
            # partition per call)
            g = gpool.tile([P, S1, D], F32)
            for s in range(S1):
                nc.gpsimd.indirect_dma_start(
                    out=g[:, s, :],
                    out_offset=None,
                    in_=table,
                    in_offset=bass.IndirectOffsetOnAxis(
                        ap=idx_all[:, t * S1 + s : t * S1 + s + 1], axis=0
                    ),
                    oob_is_err=False,
                )

            # Xself^T via PE transpose
            ps_xsT = psum.tile([P, P], F32)
            nc.tensor.transpose(ps_xsT[:], g[:, 0, :], ident[:])
            xsT = small.tile([P, P], F32)
            nc.scalar.copy(xsT[:], ps_xsT[:])

            # Q^T = Wq @ Xself^T + bq   [j, n]
            ps_qT = psum.tile([P, P], F32)
            nc.tensor.matmul(ps_qT[:], lhsT=wqT_s[:], rhs=xsT[:], start=True, stop=True)
            qT = small.tile([P, P], F32)
            nc.scalar.activation(
                qT[:],
                ps_qT[:],
                func=mybir.ActivationFunctionType.Identity,
                bias=bq_s[:, :1],
            )

            # Q' = Q @ Wk   [n, d]  (lhsT = Q^T)
            ps_qp = psum.tile([P, P], F32)
            nc.tensor.matmul(ps_qp[:], lhsT=qT[:], rhs=wk_s[:], start=True, stop=True)
            qp = small.tile([P, P], F32)
            nc.scalar.copy(qp[:], ps_qp[:])

            # scores_s[n] = sum_d G[n, s, d] * Q'[n, d]
            prod = prodp.tile([P, S1, D], F32)
            nc.vector.tensor_tensor(
                prod[:],
                g[:],
                qp[:, None, :].to_broadcast([P, S1, D]),
                op=mybir.AluOpType.mult,
            )
            sc = small.tile([P, S1], F32)
            nc.vector.tensor_reduce(
                sc[:], prod[:], axis=mybir.AxisListType.X, op=mybir.AluOpType.add
            )

            # softmax over s (free dim)
            negmax = small.tile([P, 1], F32)
            nc.vector.tensor_reduce(
                negmax[:],
                sc[:],
                axis=mybir.AxisListType.X,
                op=mybir.AluOpType.max,
                negate=True,
            )
            e = small.tile([P, S1], F32)
            zsum = small.tile([P, 1], F32)
            nc.scalar.activation(
                e[:],
                sc[:],
                func=mybir.ActivationFunctionType.Exp,
                bias=negmax[:, :1],
                accum_out=zsum[:],
            )
            zinv = small.tile([P, 1], F32)
            nc.vector.reciprocal(zinv[:], zsum[:])
            attn = small.tile([P, S1], BF16)
            nc.vector.tensor_scalar_mul(attn[:], e[:], zinv[:, :1])

            # diag_all[p, s, y] = attn[p, s] if p == y else 0  (DVE — gpsimd is
            # saturated by gather descriptor generation)
            diag = diagp.tile([P, S1, D], BF16)
            nc.vector.tensor_tensor(
                diag[:],
                ident_bf[:, None, :].to_broadcast([P, S1, D]),
                attn[:, :, None].to_broadcast([P, S1, D]),
                op=mybir.AluOpType.mult,
            )

            # bf16 copy of gathered rows for the PE weighted-sum
            gbf = gbfp.tile([P, S1, D], BF16)
            nc.scalar.copy(gbf[:], g[:])

            # Xmix^T = sum_s (G_s)^T @ diag(attn_s)   [d, n]
            ps_xm = psum_xm.tile([P, P], F32)
            for s in range(S1):
                nc.tensor.matmul(
                    ps_xm[:],
                    lhsT=gbf[:, s, :],
                    rhs=diag[:, s, :],
                    start=(s == 0),
                    stop=(s == S1 - 1),
                )
            xmT = small.tile([P, P], F32)
            nc.scalar.copy(xmT[:], ps_xm[:])

            # out^T = Wv @ Xmix^T + bv   [j, n]
            ps_mx = psum.tile([P, P], F32)
            nc.tensor.matmul(ps_mx[:], lhsT=wvT_s[:], rhs=xmT[:], start=True, stop=True)
            o_t = outp.tile([P, P], F32)
            nc.scalar.activation(
                o_t[:],
                ps_mx[:],
                func=mybir.ActivationFunctionType.Identity,
                bias=bv_s[:, :1],
            )
            nc.sync.dma_start(out[:, bass.ts(t, P)], o_t[:])

    nc.compile()
    return nc


_NC_CACHE = {}


def _get_nc():
    key = (N_TILES, VOCAB)
    if key not in _NC_CACHE:
        _NC_CACHE[key] = build_kernel()
    return _NC_CACHE[key]


def kernel(**inputs) -> np.ndarray:
    table = np.ascontiguousarray(np.asarray(inputs["table"], dtype=np.float32))
    node = np.asarray(inputs["node"]).astype(np.int32)
    neighs = np.asarray(inputs["neighs"]).astype(np.int32)
    Wq = np.asarray(inputs["Wq"], dtype=np.float32)
    bq = np.asarray(inputs["bq"], dtype=np.float32)
    Wk = np.asarray(inputs["Wk"], dtype=np.float32)
    Wv = np.asarray(inputs["Wv"], dtype=np.float32)
    bv = np.asarray(inputs["bv"], dtype=np.float32)

    idx_full = np.concatenate([node[:, None], neighs], axis=1)  # [N, S1] int32

    common = {
        "table": table,
        "wqT": np.ascontiguousarray(Wq.T),
        "wk": np.ascontiguousarray(Wk),
        "wvT": np.ascontiguousarray(Wv.T),
        "bq": np.ascontiguousarray(bq[:, None]),
        "bv": np.ascontiguousarray(bv[:, None]),
    }

    in_maps = []
    for c in range(N_CORES):
        idx_c = idx_full[c * N_PER_CORE : (c + 1) * N_PER_CORE]
        idx_pad = np.zeros((N_PAD, S1), dtype=np.int32)
        idx_pad[:N_PER_CORE] = idx_c
        in_maps.append(dict(common, idx=np.ascontiguousarray(
            idx_pad.reshape(N_TILES, P, S1).transpose(1, 0, 2).reshape(P, N_TILES * S1)
        )))

    nc = _get_nc()
    results = run_bass_kernel_spmd(nc, in_maps, list(range(N_CORES))).results

    out = np.empty((N_NODES, D), dtype=np.float32)
    for c in range(N_CORES):
        out[c * N_PER_CORE : (c + 1) * N_PER_CORE] = results[c]["out"][
            :, :N_PER_CORE
        ].T
    return out


if __name__ == "__main__":
    rng = np.random.default_rng(0)
    inputs = {
        "table": rng.standard_normal((VOCAB, D), dtype=np.float32),
        "node": rng.integers(0, VOCAB, (N_NODES,)),
        "neighs": rng.integers(0, VOCAB, (N_NODES, S)),
        "Wq": rng.uniform(-0.09, 0.09, (D, D)).astype(np.float32),
        "bq": rng.uniform(-0.09, 0.09, (D,)).astype(np.float32),
        "Wk": rng.uniform(-0.09, 0.09, (D, D)).astype(np.float32),
        "bk": rng.uniform(-0.09, 0.09, (D,)).astype(np.float32),
        "Wv": rng.uniform(-0.09, 0.09, (D, D)).astype(np.float32),
        "bv": rng.uniform(-0.09, 0.09, (D,)).astype(np.float32),
    }
    res = kernel(**inputs)
    print("kernel ran, output shape", res.shape)



# revision 3
# speedup vs baseline: 1.3837x; 1.3837x over previous
"""AttnAggregator2 Trainium2 kernel, v2: multi-queue dma_gather.

Math (per node n, rows X[s] = table rows of {self, neigh_0..24}):
    Q      = table[node] @ Wq^T + bq
    scores = Q . (X @ Wk^T + bk);  Q.bk is constant per node -> cancels in
             softmax: dropped.
    attn   = softmax(scores)
    mix    = (sum_s attn_s X_s) @ Wv^T + bv    (sum attn = 1 absorbs bv)

Gather strategy (the whole problem is gather-bound):
  - The generic indirect DMA generates descriptors on one Q7 pair at
    ~1.3us/128 rows (the baseline's bottleneck).  InstDMAGatherAnt
    (gpsimd dma_gather) generates them vectorized (~2ns/row) and runs on
    one of FOUR SWDGE queue pairs, so both descriptor generation is ~5x
    cheaper and the 16 SDMA engines overlap reads across 4 rings.
  - dma_gather indices are int16 (<32768); vocab=100000.  So the table
    (converted to bf16 on host) is split into 4 ranges of 25000 rows and
    each tile issues up to 4 gather calls, one per range, indices sorted
    by range within each node.  Per-call slot-window = max over the
    tile's 128 nodes of its range count; pad slots gather row 0 and are
    masked out of the softmax with a host-built -1e9 mask.
  - Host clusters nodes into tiles by similar range-count vectors to
    keep the per-tile max close to the mean (pad overhead ~15-25%).
  - The self row is one of the 26 sorted rows; it is recovered as
    Xself^T = sum_s G_s^T @ diag(onehot_s) on the PE (host-built onehot
    marks each node's self slot), which also yields the transposed
    layout the Q projection wants.

Sharding: data-parallel over nodes, 8 cores, table + weights replicated.
Shapes of the per-(tile,range) windows depend on the input's index
distribution, so the NEFF is built at kernel() time from the actual
inputs (compile is cached across identical inputs).
"""

import sys
from contextlib import ExitStack

import numpy as np

sys.path.insert(0, "/opt/trn_rl_repo")

import ml_dtypes

import concourse.bass as bass
import concourse.mybir as mybir
import concourse.tile as tile
from concourse import bacc
from concourse import library_config
from concourse.bass_utils import run_bass_kernel_spmd
from concourse.masks import make_identity

F32 = mybir.dt.float32
BF16 = mybir.dt.bfloat16
I16 = mybir.dt.int16

VOCAB = 100000
N_NODES = 50000
S = 25
S1 = S + 1  # self + sampled neighbors
D = 128
P = 128
N_CORES = 8
N_PER_CORE = N_NODES // N_CORES  # 6250
N_TILES = (N_PER_CORE + P - 1) // P  # 49
N_PAD = N_TILES * P  # 6272
NR = 4  # vocab ranges
RANGE = 25000  # rows per range (< 32768 so local idx fits int16)


def plan_core(idx_core: np.ndarray):
    """idx_core: [N_PAD, S1] int32 (rows of pad nodes must be -1).

    Returns dict with per-tile window sizes and packed device arrays.
    """
    npad = idx_core.shape[0]
    rng_of = idx_core // RANGE  # -1 rows -> negative
    k = np.zeros((npad, NR), dtype=np.int32)
    for r in range(NR):
        k[:, r] = (rng_of == r).sum(axis=1)

    # cluster: sort nodes by range-count vector (pad nodes k=0 sort last
    # via explicit key), tiles of 128 similar nodes
    is_pad = idx_core[:, 0] < 0
    order = np.lexsort((k[:, 3], k[:, 2], k[:, 1], k[:, 0], is_pad))
    # lexsort: last key is primary -> is_pad primary (False first), then k0...
    perm = order  # position in sorted list -> original padded-node index

    k_s = k[perm]
    m = np.zeros((N_TILES, NR), dtype=np.int32)  # slot chunks per (tile, range)
    for t in range(N_TILES):
        m[t] = k_s[t * P : (t + 1) * P].max(axis=0)
    s_tiles = m.sum(axis=1)  # S'_t per tile

    idx_cols = int((m * 8).sum())
    s_cols = int(s_tiles.sum())
    idxg = np.zeros((P, idx_cols), dtype=np.int16)
    maskv = np.full((P, s_cols), -1e9, dtype=np.float32)
    onehot = np.zeros((P, s_cols), dtype=ml_dtypes.bfloat16)

    io = 0
    so = 0
    ioff = np.zeros((N_TILES, NR), dtype=np.int64)
    soff = np.zeros(N_TILES, dtype=np.int64)
    for t in range(N_TILES):
        soff[t] = so
        rows = perm[t * P : (t + 1) * P]
        ids = idx_core[rows]  # [P, S1]
        rof = rng_of[rows]
        # per-node, per-range sorted local index lists
        for r in range(NR):
            mt = int(m[t, r])
            ioff[t, r] = io
            if mt == 0:
                continue
            blk = np.zeros((mt * P,), dtype=np.int16)  # j = s*128 + p
            for p in range(P):
                loc = ids[p][rof[p] == r] - r * RANGE
                nn = loc.shape[0]
                blk[p : nn * P + p : P] = loc.astype(np.int16)
                # mask: valid slots get 0
                maskv[p, so : so + nn] = 0.0
                if r == 0 and ids[p][0] >= 0:
                    pass  # self handled below
            # wrapped layout: j -> [j%16, j//16], replicated x8
            w = blk.reshape(mt * 8, 16).T  # [16, mt*8]
            idxg[:, io : io + mt * 8] = np.tile(w, (8, 1))
            io += mt * 8
            so += mt
        # onehot for self slot: self id = ids[p][0]; its slot = position of
        # s=0 within the sorted order = (count of rows in earlier ranges) +
        # (position within its range block).  Order within a range block is
        # the order of np.where(rof==r) i.e. original s order, so self
        # (s=0) is FIRST among its range's entries.
        for p in range(P):
            sid = ids[p][0]
            if sid < 0:
                continue
            r0 = sid // RANGE
            before = int(np.sum(m[t, :r0]))
            onehot[p, soff[t] + before] = 1.0
    assert so == s_cols and io == idx_cols
    return dict(
        perm=perm, m=m, s_tiles=s_tiles, ioff=ioff, soff=soff,
        idxg=idxg, maskv=maskv, onehot=onehot,
        idx_cols=idx_cols, s_cols=s_cols,
    )


def build_kernel(m: np.ndarray, ioff: np.ndarray, soff: np.ndarray,
                 s_tiles: np.ndarray, idx_cols: int, s_cols: int):
    nc = bacc.Bacc(
        "TRN2",
        target_bir_lowering=False,
        debug=False,
        enable_asserts=False,
        num_swdge_queues=4,
    )

    table = nc.dram_tensor("table", [VOCAB, D], BF16, kind="ExternalInput").ap()
    idxg = nc.dram_tensor("idxg", [P, idx_cols], I16, kind="ExternalInput").ap()
    maskv = nc.dram_tensor("maskv", [P, s_cols], F32, kind="ExternalInput").ap()
    onehot = nc.dram_tensor("onehot", [P, s_cols], BF16, kind="ExternalInput").ap()
    wqT = nc.dram_tensor("wqT", [D, D], F32, kind="ExternalInput").ap()
    wk = nc.dram_tensor("wk", [D, D], F32, kind="ExternalInput").ap()
    wvT = nc.dram_tensor("wvT", [D, D], F32, kind="ExternalInput").ap()
    bq = nc.dram_tensor("bq", [D, 1], F32, kind="ExternalInput").ap()
    bv = nc.dram_tensor("bv", [D, 1], F32, kind="ExternalInput").ap()
    out = nc.dram_tensor("out", [D, N_TILES * P], F32, kind="ExternalOutput").ap()

    smax = int(s_tiles.max())

    with tile.TileContext(nc) as tc, ExitStack() as ctx:
        const = ctx.enter_context(tc.tile_pool(name="const", bufs=1))
        gpool = ctx.enter_context(tc.tile_pool(name="gpool", bufs=3))
        prodp = ctx.enter_context(tc.tile_pool(name="prodp", bufs=2))
        diagp = ctx.enter_context(tc.tile_pool(name="diagp", bufs=2))
        small = ctx.enter_context(tc.tile_pool(name="small", bufs=4))
        outp = ctx.enter_context(tc.tile_pool(name="outp", bufs=3))
        psum = ctx.enter_context(tc.tile_pool(name="psum", bufs=1, space="PSUM"))
        psum_xm = ctx.enter_context(tc.tile_pool(name="psum_xm", bufs=2, space="PSUM"))
        psum_xs = ctx.enter_context(tc.tile_pool(name="psum_xs", bufs=2, space="PSUM"))

        nc.gpsimd.load_library(library_config.mlp)

        ident = const.tile([P, P], F32)
        make_identity(nc, ident[:])
        ident_bf = const.tile([P, P], BF16)
        nc.scalar.copy(ident_bf[:], ident[:])
        wqT_s = const.tile([D, D], F32)
        nc.sync.dma_start(wqT_s[:], wqT)
        wk_s = const.tile([D, D], F32)
        nc.sync.dma_start(wk_s[:], wk)
        wvT_s = const.tile([D, D], F32)
        nc.sync.dma_start(wvT_s[:], wvT)
        bq_s = const.tile([D, 1], F32)
        nc.sync.dma_start(bq_s[:], bq)
        bv_s = const.tile([D, 1], F32)
        nc.sync.dma_start(bv_s[:], bv)
        idx_all = const.tile([P, idx_cols], I16)
        nc.sync.dma_start(idx_all[:], idxg)
        mask_all = const.tile([P, s_cols], F32)
        nc.sync.dma_start(mask_all[:], maskv)
        oh_all = const.tile([P, s_cols], BF16)
        nc.sync.dma_start(oh_all[:], onehot)

        for t in range(N_TILES):
            st = int(s_tiles[t])
            so = int(soff[t])
            g = gpool.tile([P, st, D], BF16)
            wo = 0
            for r in range(NR):
                mt = int(m[t, r])
                if mt == 0:
                    continue
                io = int(ioff[t, r])
                nc.gpsimd.dma_gather(
                    g[:, wo : wo + mt, :],
                    table[r * RANGE : (r + 1) * RANGE, :],
                    idx_all[:, io : io + mt * 8],
                    mt * P,
                    mt * P,
                    D,
                    single_packet=False,
                    queue_num=(t + r) % 4,
                )
                wo += mt

            # Xself[n, d] = sum_s G[n, s, d] * onehot[n, s]  (DVE, transposed
            # iteration so the reduce is over the innermost axis)
            prod2 = prodp.tile([P, D, st], F32)
            nc.vector.tensor_tensor(
                prod2[:],
                g[:].transpose([0, 2, 1]),
                oh_all[:, so : so + st, None].to_broadcast([P, st, D]).transpose(
                    [0, 2, 1]
                ),
                op=mybir.AluOpType.mult,
            )
            xself = small.tile([P, D], F32)
            nc.vector.tensor_reduce(
                xself[:], prod2[:], axis=mybir.AxisListType.X, op=mybir.AluOpType.add
            )
            # Xself^T via PE transpose
            ps_xs = psum_xs.tile([P, P], F32)
            nc.tensor.transpose(ps_xs[:], xself[:], ident[:])
            xsT = small.tile([P, P], F32)
            nc.scalar.copy(xsT[:], ps_xs[:])

            # Q^T = Wq @ Xself^T + bq   [j, n]
            ps_qT = psum.tile([P, P], F32)
            nc.tensor.matmul(ps_qT[:], lhsT=wqT_s[:], rhs=xsT[:], start=True, stop=True)
            qT = small.tile([P, P], F32)
            nc.scalar.activation(
                qT[:], ps_qT[:],
                func=mybir.ActivationFunctionType.Identity,
                bias=bq_s[:, :1],
            )

            # Q' = Q @ Wk   [n, d]
            ps_qp = psum.tile([P, P], F32)
            nc.tensor.matmul(ps_qp[:], lhsT=qT[:], rhs=wk_s[:], start=True, stop=True)
            qp = small.tile([P, P], BF16)
            nc.scalar.copy(qp[:], ps_qp[:])

            # scores_s[n] = sum_d G[n, s, d] * Q'[n, d]  + mask
            prod = prodp.tile([P, st, D], F32)
            nc.vector.tensor_tensor(
                prod[:], g[:],
                qp[:, None, :].to_broadcast([P, st, D]),
                op=mybir.AluOpType.mult,
            )
            sc = small.tile([P, st], F32)
            nc.vector.tensor_reduce(
                sc[:], prod[:], axis=mybir.AxisListType.X, op=mybir.AluOpType.add
            )
            scm = small.tile([P, st], F32)
            nc.vector.tensor_tensor(
                scm[:], sc[:], mask_all[:, so : so + st], op=mybir.AluOpType.add
            )

            # softmax over s
            negmax = small.tile([P, 1], F32)
            nc.vector.tensor_reduce(
                negmax[:], scm[:], axis=mybir.AxisListType.X,
                op=mybir.AluOpType.max, negate=True,
            )
            e = small.tile([P, st], F32)
            zsum = small.tile([P, 1], F32)
            nc.scalar.activation(
                e[:], scm[:],
                func=mybir.ActivationFunctionType.Exp,
                bias=negmax[:, :1],
                accum_out=zsum[:],
            )
            zinv = small.tile([P, 1], F32)
            nc.vector.reciprocal(zinv[:], zsum[:])
            attn = small.tile([P, st], BF16)
            nc.vector.tensor_scalar_mul(attn[:], e[:], zinv[:, :1])

            # diag_s = diag(attn[:, s])
            diag = diagp.tile([P, st, D], BF16)
            nc.vector.tensor_tensor(
                diag[:],
                ident_bf[:, None, :].to_broadcast([P, st, D]),
                attn[:, :, None].to_broadcast([P, st, D]),
                op=mybir.AluOpType.mult,
            )

            # Xmix^T = sum_s G_s^T @ diag_s   [d, n]
            ps_xm = psum_xm.tile([P, P], F32)
            for s in range(st):
                nc.tensor.matmul(
                    ps_xm[:], lhsT=g[:, s, :], rhs=diag[:, s, :],
                    start=(s == 0), stop=(s == st - 1),
                )
            xmT = small.tile([P, P], F32)
            nc.scalar.copy(xmT[:], ps_xm[:])

            # out^T = Wv @ Xmix^T + bv   [j, n]
            ps_mx = psum.tile([P, P], F32)
            nc.tensor.matmul(ps_mx[:], lhsT=wvT_s[:], rhs=xmT[:], start=True, stop=True)
            o_t = outp.tile([P, P], F32)
            nc.scalar.activation(
                o_t[:], ps_mx[:],
                func=mybir.ActivationFunctionType.Identity,
                bias=bv_s[:, :1],
            )
            nc.sync.dma_start(out[:, bass.ts(t, P)], o_t[:])

    nc.compile()
    return nc


_CACHE = {}


def prepare(inputs):
    """Returns (nc, in_maps, perms) ready for run_bass_kernel_spmd."""
    table = np.asarray(inputs["table"], dtype=np.float32)
    node = np.asarray(inputs["node"]).astype(np.int64)
    neighs = np.asarray(inputs["neighs"]).astype(np.int64)
    Wq = np.asarray(inputs["Wq"], dtype=np.float32)
    bq = np.asarray(inputs["bq"], dtype=np.float32)
    Wk = np.asarray(inputs["Wk"], dtype=np.float32)
    Wv = np.asarray(inputs["Wv"], dtype=np.float32)
    bv = np.asarray(inputs["bv"], dtype=np.float32)

    table_bf = np.ascontiguousarray(table.astype(ml_dtypes.bfloat16))
    idx_full = np.concatenate([node[:, None], neighs], axis=1).astype(np.int32)

    common = {
        "table": table_bf,
        "wqT": np.ascontiguousarray(Wq.T),
        "wk": np.ascontiguousarray(Wk),
        "wvT": np.ascontiguousarray(Wv.T),
        "bq": np.ascontiguousarray(bq[:, None]),
        "bv": np.ascontiguousarray(bv[:, None]),
    }

    plans = []
    in_maps = []
    for c in range(N_CORES):
        idx_c = idx_full[c * N_PER_CORE : (c + 1) * N_PER_CORE]
        idx_pad = np.full((N_PAD, S1), -1, dtype=np.int32)
        idx_pad[:N_PER_CORE] = idx_c
        pl = plan_core(idx_pad)
        plans.append(pl)
        in_maps.append(dict(
            common,
            idxg=np.ascontiguousarray(pl["idxg"]),
            maskv=np.ascontiguousarray(pl["maskv"]),
            onehot=np.ascontiguousarray(pl["onehot"]),
        ))

    # all cores must share one NEFF: use the elementwise max window sizes
    # across cores so a single build serves all (per-core arrays are padded
    # up to the common shape)
    m_all = np.stack([pl["m"] for pl in plans])  # [C, T, R]
    m_max = m_all.max(axis=0)
    s_tiles = m_max.sum(axis=1)
    ioff = np.zeros((N_TILES, NR), dtype=np.int64)
    soff = np.zeros(N_TILES, dtype=np.int64)
    io = 0
    so = 0
    for t in range(N_TILES):
        soff[t] = so
        for r in range(NR):
            ioff[t, r] = io
            io += int(m_max[t, r]) * 8
        so += int(s_tiles[t])
    idx_cols, s_cols = io, so

    # repack each core's arrays into the common layout
    for c in range(N_CORES):
        pl = plans[c]
        idxg = np.zeros((P, idx_cols), dtype=np.int16)
        maskv = np.full((P, s_cols), -1e9, dtype=np.float32)
        onehot = np.zeros((P, s_cols), dtype=ml_dtypes.bfloat16)
        for t in range(N_TILES):
            so_c = int(pl["soff"][t])
            wo_new = int(soff[t])
            for r in range(NR):
                mc = int(pl["m"][t, r])
                mn = int(m_max[t, r])
                if mc > 0:
                    io_c = int(pl["ioff"][t, r])
                    io_n = int(ioff[t, r])
                    idxg[:, io_n : io_n + mc * 8] = pl["idxg"][:, io_c : io_c + mc * 8]
                    maskv[:, wo_new : wo_new + mc] = pl["maskv"][:, so_c : so_c + mc]
                    onehot[:, wo_new : wo_new + mc] = pl["onehot"][:, so_c : so_c + mc]
                so_c += mc
                wo_new += mn
        in_maps[c]["idxg"] = np.ascontiguousarray(idxg)
        in_maps[c]["maskv"] = np.ascontiguousarray(maskv)
        in_maps[c]["onehot"] = np.ascontiguousarray(onehot)

    key = (idx_cols, s_cols, m_max.tobytes())
    if key not in _CACHE:
        _CACHE[key] = build_kernel(m_max, ioff, soff, s_tiles, idx_cols, s_cols)
    nc = _CACHE[key]
    perms = [pl["perm"] for pl in plans]
    return nc, in_maps, perms


def kernel(**inputs) -> np.ndarray:
    nc, in_maps, perms = prepare(inputs)
    results = run_bass_kernel_spmd(nc, in_maps, list(range(N_CORES))).results

    out = np.empty((N_NODES, D), dtype=np.float32)
    for c in range(N_CORES):
        o_t = results[c]["out"]  # [D, N_PAD] transposed, in permuted order
        o = o_t.T  # [N_PAD, D] rows follow perm order
        inv = perms[c]  # sorted position -> padded node index
        full = np.empty((N_PAD, D), dtype=np.float32)
        full[inv] = o
        out[c * N_PER_CORE : (c + 1) * N_PER_CORE] = full[:N_PER_CORE]
    return out


if __name__ == "__main__":
    rng = np.random.default_rng(0)
    inputs = {
        "table": rng.standard_normal((VOCAB, D), dtype=np.float32),
        "node": rng.integers(0, VOCAB, (N_NODES,)),
        "neighs": rng.integers(0, VOCAB, (N_NODES, S)),
        "Wq": rng.uniform(-0.09, 0.09, (D, D)).astype(np.float32),
        "bq": rng.uniform(-0.09, 0.09, (D,)).astype(np.float32),
        "Wk": rng.uniform(-0.09, 0.09, (D, D)).astype(np.float32),
        "bk": rng.uniform(-0.09, 0.09, (D,)).astype(np.float32),
        "Wv": rng.uniform(-0.09, 0.09, (D, D)).astype(np.float32),
        "bv": rng.uniform(-0.09, 0.09, (D,)).astype(np.float32),
    }
    res = kernel(**inputs)
    print("kernel ran, output shape", res.shape)


# revision 4
# speedup vs baseline: 1.6352x; 1.1817x over previous
"""AttnAggregator2 Trainium2 kernel, v2: multi-queue dma_gather (f32 rows).

Gather strategy (the problem is gather-bound):
  - The generic indirect DMA generates descriptors on one Q7 pair at
    ~1.3us/128 rows (the baseline bottleneck).  InstDMAGatherAnt
    (gpsimd dma_gather) generates them vectorized (~2ns/row) and its
    queue_num routes work to one of FOUR SWDGE queue pairs, so the 16
    SDMA engines overlap HBM reads across 4 descriptor rings (~3.4x
    drain speedup).
  - dma_gather indices are int16 (<32768); vocab=100000.  The table is
    split into 4 ranges of 25000 rows; each tile issues up to 4 gather
    calls, one per range, neighbor indices sorted by range within each
    node.  Per-call slot window = max over the tile's 128 nodes of the
    node's range count; pad slots gather row 0 and are masked out of
    the softmax with a host-built -1e9 mask.  The host clusters nodes
    into tiles by similar range-count vectors to keep the windows tight.
  - Self features (table[node], 1/26 of lookups) are host-gathered and
    uploaded dense: f32 transposed for the Q projection, bf16 row-major
    for the self score slot + PE weighted sum.

Per-core math per tile (window slots st, score slots 1+st):
    Q^T   = Wq @ Xself^T + bq            (PE, from uploaded Xself^T)
    Q'    = Q @ Wk                        (PE)
    sc    = [Xself.Q' | reduce_d(G*Q')] + mask ; attn = softmax(sc)
    Xmix^T= Xself^T@diag(a_0) + sum_s G_s^T@diag(a_s)   (PE, bf16)
    out^T = Wv @ Xmix^T + bv
Window shapes depend on the input index distribution, so the NEFF is
built at kernel() time from the actual inputs (compile cached).

Sharding: data-parallel over nodes, 8 cores, table + weights replicated.
"""

import sys
from contextlib import ExitStack

import numpy as np

sys.path.insert(0, "/opt/trn_rl_repo")

import ml_dtypes

import concourse.bass as bass
import concourse.mybir as mybir
import concourse.tile as tile
from concourse import bacc
from concourse import library_config
from concourse.bass_utils import run_bass_kernel_spmd
from concourse.masks import make_identity

F32 = mybir.dt.float32
BF16 = mybir.dt.bfloat16
I16 = mybir.dt.int16

VOCAB = 100000
N_NODES = 50000
S = 25  # sampled neighbors (self handled separately)
D = 128
P = 128
N_CORES = 8
N_PER_CORE = N_NODES // N_CORES  # 6250
N_TILES = (N_PER_CORE + P - 1) // P  # 49
N_PAD = N_TILES * P  # 6272
NR = 4
RANGE = 25000  # rows per table range (< 32768 so local idx fits int16)


def plan_core(neigh_core: np.ndarray):
    """neigh_core: [N_PAD, S] int32 neighbor ids (pad nodes rows = -1)."""
    npad = neigh_core.shape[0]
    rng_of = neigh_core // RANGE  # -1 -> negative
    k = np.zeros((npad, NR), dtype=np.int32)
    for r in range(NR):
        k[:, r] = (rng_of == r).sum(axis=1)

    is_pad = neigh_core[:, 0] < 0
    order = np.lexsort((k[:, 3], k[:, 2], k[:, 1], k[:, 0], is_pad))
    perm = order

    k_s = k[perm]
    m = np.zeros((N_TILES, NR), dtype=np.int32)
    for t in range(N_TILES):
        m[t] = k_s[t * P : (t + 1) * P].max(axis=0)
    return dict(perm=perm, m=m, k=k, rng_of=rng_of)


def pack_core(pl, neigh_core, m_all, ioff, soff, idx_cols, s_cols):
    """Build packed idx/mask arrays in the common (max-window) layout."""
    perm = pl["perm"]
    rng_of = pl["rng_of"]
    idxg = np.zeros((P, idx_cols), dtype=np.int16)
    maskv = np.full((P, s_cols), -1e9, dtype=np.float32)
    for t in range(N_TILES):
        rows = perm[t * P : (t + 1) * P]
        ids = neigh_core[rows]
        rof = rng_of[rows]
        wo = int(soff[t]) + 1  # col 0 of each tile's score block = self
        for r in range(NR):
            mt = int(m_all[t, r])
            if mt == 0:
                continue
            io = int(ioff[t, r])
            blk = np.zeros((mt * P,), dtype=np.int16)
            for p in range(P):
                loc = ids[p][rof[p] == r] - r * RANGE
                nn = loc.shape[0]
                blk[p : nn * P + p : P] = loc.astype(np.int16)
                maskv[p, wo : wo + nn] = 0.0
            w = blk.reshape(mt * 8, 16).T
            idxg[:, io : io + mt * 8] = np.tile(w, (8, 1))
            wo += mt
        # self col valid unless pad node
        for p in range(P):
            if neigh_core[rows[p], 0] >= 0:
                maskv[p, int(soff[t])] = 0.0
    return idxg, maskv


def build_kernel(m: np.ndarray, ioff: np.ndarray, soff: np.ndarray,
                 s_tiles: np.ndarray, idx_cols: int, s_cols: int):
    nc = bacc.Bacc(
        "TRN2",
        target_bir_lowering=False,
        debug=False,
        enable_asserts=False,
        num_swdge_queues=4,
    )

    table = nc.dram_tensor("table", [VOCAB, D], F32, kind="ExternalInput").ap()
    idxg = nc.dram_tensor("idxg", [P, idx_cols], I16, kind="ExternalInput").ap()
    maskv = nc.dram_tensor("maskv", [P, s_cols], F32, kind="ExternalInput").ap()
    xsT_d = nc.dram_tensor("xsT", [D, N_TILES * P], F32, kind="ExternalInput").ap()
    xsbf_d = nc.dram_tensor("xsbf", [P, N_TILES * D], BF16, kind="ExternalInput").ap()
    wqT = nc.dram_tensor("wqT", [D, D], F32, kind="ExternalInput").ap()
    wk = nc.dram_tensor("wk", [D, D], F32, kind="ExternalInput").ap()
    wvT = nc.dram_tensor("wvT", [D, D], F32, kind="ExternalInput").ap()
    bq = nc.dram_tensor("bq", [D, 1], F32, kind="ExternalInput").ap()
    bv = nc.dram_tensor("bv", [D, 1], F32, kind="ExternalInput").ap()
    out = nc.dram_tensor("out", [D, N_TILES * P], F32, kind="ExternalOutput").ap()

    with tile.TileContext(nc) as tc, ExitStack() as ctx:
        const = ctx.enter_context(tc.tile_pool(name="const", bufs=1))
        gpool = ctx.enter_context(tc.tile_pool(name="gpool", bufs=2))
        gbfp = ctx.enter_context(tc.tile_pool(name="gbfp", bufs=2))
        prodp = ctx.enter_context(tc.tile_pool(name="prodp", bufs=1))
        diagp = ctx.enter_context(tc.tile_pool(name="diagp", bufs=2))
        small = ctx.enter_context(tc.tile_pool(name="small", bufs=6))
        outp = ctx.enter_context(tc.tile_pool(name="outp", bufs=3))
        xsp = ctx.enter_context(tc.tile_pool(name="xsp", bufs=3))
        psum = ctx.enter_context(tc.tile_pool(name="psum", bufs=1, space="PSUM"))
        psum_xm = ctx.enter_context(tc.tile_pool(name="psum_xm", bufs=2, space="PSUM"))

        nc.gpsimd.load_library(library_config.mlp)

        ident = const.tile([P, P], F32)
        make_identity(nc, ident[:])
        ident_bf = const.tile([P, P], BF16)
        nc.scalar.copy(ident_bf[:], ident[:])
        wqT_s = const.tile([D, D], F32)
        nc.sync.dma_start(wqT_s[:], wqT)
        wk_s = const.tile([D, D], F32)
        nc.sync.dma_start(wk_s[:], wk)
        wvT_s = const.tile([D, D], F32)
        nc.sync.dma_start(wvT_s[:], wvT)
        bq_s = const.tile([D, 1], F32)
        nc.sync.dma_start(bq_s[:], bq)
        bv_s = const.tile([D, 1], F32)
        nc.sync.dma_start(bv_s[:], bv)
        idx_all = const.tile([P, idx_cols], I16)
        nc.sync.dma_start(idx_all[:], idxg)
        mask_all = const.tile([P, s_cols], F32)
        nc.sync.dma_start(mask_all[:], maskv)
        xsbf_all = const.tile([P, N_TILES * D], BF16)
        nc.sync.dma_start(xsbf_all[:], xsbf_d)

        for t in range(N_TILES):
            st = int(s_tiles[t])  # neighbor window slots
            ss = st + 1  # score slots incl self at col 0
            so = int(soff[t])
            xsT = xsp.tile([D, P], F32)
            nc.sync.dma_start(xsT[:], xsT_d[:, bass.ts(t, P)])
            xsbf = xsbf_all[:, bass.ts(t, D)]

            g = gpool.tile([P, st, D], F32)
            wo = 0
            for r in range(NR):
                mt = int(m[t, r])
                if mt == 0:
                    continue
                io = int(ioff[t, r])
                nc.gpsimd.dma_gather(
                    g[:, wo : wo + mt, :],
                    table[r * RANGE : (r + 1) * RANGE, :],
                    idx_all[:, io : io + mt * 8],
                    mt * P,
                    mt * P,
                    D,
                    single_packet=False,
                    queue_num=(t + r) % 4,
                )
                wo += mt

            # Q^T = Wq @ Xself^T + bq   [j, n]
            ps_qT = psum.tile([P, P], F32)
            nc.tensor.matmul(ps_qT[:], lhsT=wqT_s[:], rhs=xsT[:], start=True, stop=True)
            qT = small.tile([P, P], F32)
            nc.scalar.activation(
                qT[:], ps_qT[:],
                func=mybir.ActivationFunctionType.Identity,
                bias=bq_s[:, :1],
            )

            # Q' = Q @ Wk   [n, d]
            ps_qp = psum.tile([P, P], F32)
            nc.tensor.matmul(ps_qp[:], lhsT=qT[:], rhs=wk_s[:], start=True, stop=True)
            qp = small.tile([P, P], F32)
            nc.scalar.copy(qp[:], ps_qp[:])
            qp_bf = small.tile([P, P], BF16)
            nc.scalar.copy(qp_bf[:], ps_qp[:])

            # scores: col 0 self, cols 1..ss neighbors
            sc = small.tile([P, ss], F32)
            prods = small.tile([P, D], F32)
            nc.vector.tensor_tensor(
                prods[:], xsbf, qp_bf[:], op=mybir.AluOpType.mult
            )
            nc.vector.tensor_reduce(
                sc[:, 0:1], prods[:], axis=mybir.AxisListType.X,
                op=mybir.AluOpType.add,
            )
            prod = prodp.tile([P, st, D], F32)
            nc.vector.tensor_tensor(
                prod[:], g[:],
                qp[:, None, :].to_broadcast([P, st, D]),
                op=mybir.AluOpType.mult,
            )
            nc.vector.tensor_reduce(
                sc[:, 1:ss], prod[:], axis=mybir.AxisListType.X,
                op=mybir.AluOpType.add,
            )
            scm = small.tile([P, ss], F32)
            nc.vector.tensor_tensor(
                scm[:], sc[:], mask_all[:, so : so + ss], op=mybir.AluOpType.add
            )

            # softmax over score slots
            negmax = small.tile([P, 1], F32)
            nc.vector.tensor_reduce(
                negmax[:], scm[:], axis=mybir.AxisListType.X,
                op=mybir.AluOpType.max, negate=True,
            )
            e = small.tile([P, ss], F32)
            zsum = small.tile([P, 1], F32)
            nc.scalar.activation(
                e[:], scm[:],
                func=mybir.ActivationFunctionType.Exp,
                bias=negmax[:, :1],
                accum_out=zsum[:],
            )
            zinv = small.tile([P, 1], F32)
            nc.vector.reciprocal(zinv[:], zsum[:])
            attn = small.tile([P, ss], BF16)
            nc.vector.tensor_scalar_mul(attn[:], e[:], zinv[:, :1])

            # diag_s = diag(attn[:, s])  [P, ss, D]
            diag = diagp.tile([P, ss, D], BF16)
            nc.vector.tensor_tensor(
                diag[:],
                ident_bf[:, None, :].to_broadcast([P, ss, D]),
                attn[:, :, None].to_broadcast([P, ss, D]),
                op=mybir.AluOpType.mult,
            )

            gbf = gbfp.tile([P, st, D], BF16)
            nc.scalar.copy(gbf[:], g[:])

            # Xmix^T = Xself^T@diag_0 + sum_s G_s^T @ diag_{s+1}
            ps_xm = psum_xm.tile([P, P], F32)
            nc.tensor.matmul(
                ps_xm[:], lhsT=xsbf, rhs=diag[:, 0, :], start=True, stop=False
            )
            for s in range(st):
                nc.tensor.matmul(
                    ps_xm[:], lhsT=gbf[:, s, :], rhs=diag[:, s + 1, :],
                    start=False, stop=(s == st - 1),
                )
            xmT = small.tile([P, P], F32)
            nc.scalar.copy(xmT[:], ps_xm[:])

            # out^T = Wv @ Xmix^T + bv   [j, n]
            ps_mx = psum.tile([P, P], F32)
            nc.tensor.matmul(ps_mx[:], lhsT=wvT_s[:], rhs=xmT[:], start=True, stop=True)
            o_t = outp.tile([P, P], F32)
            nc.scalar.activation(
                o_t[:], ps_mx[:],
                func=mybir.ActivationFunctionType.Identity,
                bias=bv_s[:, :1],
            )
            nc.sync.dma_start(out[:, bass.ts(t, P)], o_t[:])

    nc.compile()
    return nc


_CACHE = {}


def prepare(inputs):
    table = np.ascontiguousarray(np.asarray(inputs["table"], dtype=np.float32))
    node = np.asarray(inputs["node"]).astype(np.int64)
    neighs = np.asarray(inputs["neighs"]).astype(np.int64)
    Wq = np.asarray(inputs["Wq"], dtype=np.float32)
    bq = np.asarray(inputs["bq"], dtype=np.float32)
    Wk = np.asarray(inputs["Wk"], dtype=np.float32)
    Wv = np.asarray(inputs["Wv"], dtype=np.float32)
    bv = np.asarray(inputs["bv"], dtype=np.float32)

    common = {
        "table": table,
        "wqT": np.ascontiguousarray(Wq.T),
        "wk": np.ascontiguousarray(Wk),
        "wvT": np.ascontiguousarray(Wv.T),
        "bq": np.ascontiguousarray(bq[:, None]),
        "bv": np.ascontiguousarray(bv[:, None]),
    }

    plans = []
    neighs_pad = []
    nodes_pad = []
    for c in range(N_CORES):
        nb = np.full((N_PAD, S), -1, dtype=np.int64)
        nb[:N_PER_CORE] = neighs[c * N_PER_CORE : (c + 1) * N_PER_CORE]
        nd = np.zeros((N_PAD,), dtype=np.int64)
        nd[:N_PER_CORE] = node[c * N_PER_CORE : (c + 1) * N_PER_CORE]
        neighs_pad.append(nb)
        nodes_pad.append(nd)
        plans.append(plan_core(nb.astype(np.int32)))

    m_all = np.stack([pl["m"] for pl in plans]).max(axis=0)  # [T, R]
    s_tiles = m_all.sum(axis=1)
    ioff = np.zeros((N_TILES, NR), dtype=np.int64)
    soff = np.zeros(N_TILES, dtype=np.int64)
    io = 0
    so = 0
    for t in range(N_TILES):
        soff[t] = so
        for r in range(NR):
            ioff[t, r] = io
            io += int(m_all[t, r]) * 8
        so += int(s_tiles[t]) + 1  # +1 self col
    idx_cols, s_cols = io, so

    in_maps = []
    for c in range(N_CORES):
        pl = plans[c]
        idxg, maskv = pack_core(
            pl, neighs_pad[c].astype(np.int32), m_all, ioff, soff, idx_cols, s_cols
        )
        nd_sorted = nodes_pad[c][pl["perm"]]
        xs = table[nd_sorted]  # [N_PAD, D] f32 (pad nodes -> row of node 0: fine)
        xsT = np.ascontiguousarray(xs.T)  # [D, N_PAD]
        xsbf = np.ascontiguousarray(
            xs.reshape(N_TILES, P, D).transpose(1, 0, 2).reshape(P, N_TILES * D)
            .astype(ml_dtypes.bfloat16)
        )
        in_maps.append(dict(
            common,
            idxg=np.ascontiguousarray(idxg),
            maskv=np.ascontiguousarray(maskv),
            xsT=xsT,
            xsbf=xsbf,
        ))

    key = (idx_cols, s_cols, m_all.tobytes())
    if key not in _CACHE:
        _CACHE[key] = build_kernel(m_all, ioff, soff, s_tiles, idx_cols, s_cols)
    nc = _CACHE[key]
    perms = [pl["perm"] for pl in plans]
    return nc, in_maps, perms


def kernel(**inputs) -> np.ndarray:
    nc, in_maps, perms = prepare(inputs)
    results = run_bass_kernel_spmd(nc, in_maps, list(range(N_CORES))).results

    out = np.empty((N_NODES, D), dtype=np.float32)
    for c in range(N_CORES):
        o = results[c]["out"].T  # [N_PAD, D] rows in sorted order
        full = np.empty((N_PAD, D), dtype=np.float32)
        full[perms[c]] = o
        out[c * N_PER_CORE : (c + 1) * N_PER_CORE] = full[:N_PER_CORE]
    return out


if __name__ == "__main__":
    rng = np.random.default_rng(0)
    inputs = {
        "table": rng.standard_normal((VOCAB, D), dtype=np.float32),
        "node": rng.integers(0, VOCAB, (N_NODES,)),
        "neighs": rng.integers(0, VOCAB, (N_NODES, S)),
        "Wq": rng.uniform(-0.09, 0.09, (D, D)).astype(np.float32),
        "bq": rng.uniform(-0.09, 0.09, (D,)).astype(np.float32),
        "Wk": rng.uniform(-0.09, 0.09, (D, D)).astype(np.float32),
        "bk": rng.uniform(-0.09, 0.09, (D,)).astype(np.float32),
        "Wv": rng.uniform(-0.09, 0.09, (D, D)).astype(np.float32),
        "bv": rng.uniform(-0.09, 0.09, (D,)).astype(np.float32),
    }
    res = kernel(**inputs)
    print("kernel ran, output shape", res.shape)


# revision 5
# speedup vs baseline: 1.6880x; 1.0323x over previous
"""AttnAggregator2 Trainium2 kernel, v2: multi-queue dma_gather (f32 rows).

Gather strategy (the problem is gather-bound):
  - The generic indirect DMA generates descriptors on one Q7 pair at
    ~1.3us/128 rows (the baseline bottleneck).  InstDMAGatherAnt
    (gpsimd dma_gather) generates them vectorized (~2ns/row) and its
    queue_num routes work to one of FOUR SWDGE queue pairs, so the 16
    SDMA engines overlap HBM reads across 4 descriptor rings (~3.4x
    drain speedup).
  - dma_gather indices are int16 (<32768); vocab=100000.  The table is
    split into 4 ranges of 25000 rows; each tile issues up to 4 gather
    calls, one per range, neighbor indices sorted by range within each
    node.  Per-call slot window = max over the tile's 128 nodes of the
    node's range count; pad slots gather row 0 and are masked out of
    the softmax with a host-built -1e9 mask.  The host clusters nodes
    into tiles by similar range-count vectors to keep the windows tight.
  - Self features (table[node], 1/26 of lookups) are host-gathered and
    uploaded dense: f32 transposed for the Q projection, bf16 row-major
    for the self score slot + PE weighted sum.

Per-core math per tile (window slots st, score slots 1+st):
    Q^T   = Wq @ Xself^T + bq            (PE, from uploaded Xself^T)
    Q'    = Q @ Wk                        (PE)
    sc    = [Xself.Q' | reduce_d(G*Q')] + mask ; attn = softmax(sc)
    Xmix^T= Xself^T@diag(a_0) + sum_s G_s^T@diag(a_s)   (PE, bf16)
    out^T = Wv @ Xmix^T + bv
Window shapes depend on the input index distribution, so the NEFF is
built at kernel() time from the actual inputs (compile cached).

Sharding: data-parallel over nodes, 8 cores, table + weights replicated.
"""

import sys
from contextlib import ExitStack

import numpy as np

sys.path.insert(0, "/opt/trn_rl_repo")

import ml_dtypes

import concourse.bass as bass
import concourse.mybir as mybir
import concourse.tile as tile
from concourse import bacc
from concourse import library_config
from concourse.bass_utils import run_bass_kernel_spmd
from concourse.masks import make_identity

F32 = mybir.dt.float32
BF16 = mybir.dt.bfloat16
I16 = mybir.dt.int16

VOCAB = 100000
N_NODES = 50000
S = 25  # sampled neighbors (self handled separately)
D = 128
P = 128
N_CORES = 8
N_PER_CORE = N_NODES // N_CORES  # 6250
N_TILES = (N_PER_CORE + P - 1) // P  # 49
N_PAD = N_TILES * P  # 6272
NR = 4
RANGE = 25000  # rows per table range (< 32768 so local idx fits int16)


N_GLOBAL_PAD = N_PAD * N_CORES  # 50176


def plan_global(neighs: np.ndarray):
    """Global clustering: sort all nodes by range-count vector, deal
    1024-blocks across (tile, core) so every core's tile t shares a tight
    window profile."""
    nb = np.full((N_GLOBAL_PAD, S), -1, dtype=np.int32)
    nb[:N_NODES] = neighs.astype(np.int32)
    rng_of = nb // RANGE
    k = np.zeros((N_GLOBAL_PAD, NR), dtype=np.int32)
    for r in range(NR):
        k[:, r] = (rng_of == r).sum(axis=1)
    is_pad = nb[:, 0] < 0
    order = np.lexsort((k[:, 3], k[:, 2], k[:, 1], k[:, 0], is_pad))

    perms = []
    for c in range(N_CORES):
        pc = np.empty(N_PAD, dtype=np.int64)
        for t in range(N_TILES):
            blk = order[t * P * N_CORES : (t + 1) * P * N_CORES]
            pc[t * P : (t + 1) * P] = blk[c * P : (c + 1) * P]
        perms.append(pc)

    m = np.zeros((N_TILES, NR), dtype=np.int32)
    for t in range(N_TILES):
        blk = order[t * P * N_CORES : (t + 1) * P * N_CORES]
        m[t] = k[blk].max(axis=0)
    return perms, m, nb, rng_of


def pack_core(perm, neigh_core, rng_of, m_all, ioff, soff, idx_cols, s_cols):
    """Build packed idx/mask arrays in the common (max-window) layout."""
    idxg = np.zeros((P, idx_cols), dtype=np.int16)
    maskv = np.full((P, s_cols), -1e9, dtype=np.float32)
    for t in range(N_TILES):
        rows = perm[t * P : (t + 1) * P]
        ids = neigh_core[rows]
        rof = rng_of[rows]
        wo = int(soff[t]) + 1  # col 0 of each tile's score block = self
        for r in range(NR):
            mt = int(m_all[t, r])
            if mt == 0:
                continue
            io = int(ioff[t, r])
            blk = np.zeros((mt * P,), dtype=np.int16)
            for p in range(P):
                loc = ids[p][rof[p] == r] - r * RANGE
                nn = loc.shape[0]
                blk[p : nn * P + p : P] = loc.astype(np.int16)
                maskv[p, wo : wo + nn] = 0.0
            w = blk.reshape(mt * 8, 16).T
            idxg[:, io : io + mt * 8] = np.tile(w, (8, 1))
            wo += mt
        # self col valid unless pad node
        for p in range(P):
            if neigh_core[rows[p], 0] >= 0:
                maskv[p, int(soff[t])] = 0.0
    return idxg, maskv


def build_kernel(m: np.ndarray, ioff: np.ndarray, soff: np.ndarray,
                 s_tiles: np.ndarray, idx_cols: int, s_cols: int):
    nc = bacc.Bacc(
        "TRN2",
        target_bir_lowering=False,
        debug=False,
        enable_asserts=False,
        num_swdge_queues=4,
    )

    table = nc.dram_tensor("table", [VOCAB, D], F32, kind="ExternalInput").ap()
    idxg = nc.dram_tensor("idxg", [P, idx_cols], I16, kind="ExternalInput").ap()
    maskv = nc.dram_tensor("maskv", [P, s_cols], F32, kind="ExternalInput").ap()
    xsT_d = nc.dram_tensor("xsT", [D, N_TILES * P], F32, kind="ExternalInput").ap()
    xsbf_d = nc.dram_tensor("xsbf", [P, N_TILES * D], BF16, kind="ExternalInput").ap()
    wqT = nc.dram_tensor("wqT", [D, D], F32, kind="ExternalInput").ap()
    wk = nc.dram_tensor("wk", [D, D], F32, kind="ExternalInput").ap()
    wvT = nc.dram_tensor("wvT", [D, D], F32, kind="ExternalInput").ap()
    bq = nc.dram_tensor("bq", [D, 1], F32, kind="ExternalInput").ap()
    bv = nc.dram_tensor("bv", [D, 1], F32, kind="ExternalInput").ap()
    out = nc.dram_tensor("out", [D, N_TILES * P], F32, kind="ExternalOutput").ap()

    with tile.TileContext(nc) as tc, ExitStack() as ctx:
        const = ctx.enter_context(tc.tile_pool(name="const", bufs=1))
        gpool = ctx.enter_context(tc.tile_pool(name="gpool", bufs=2))
        gbfp = ctx.enter_context(tc.tile_pool(name="gbfp", bufs=2))
        prodp = ctx.enter_context(tc.tile_pool(name="prodp", bufs=1))
        diagp = ctx.enter_context(tc.tile_pool(name="diagp", bufs=2))
        small = ctx.enter_context(tc.tile_pool(name="small", bufs=6))
        outp = ctx.enter_context(tc.tile_pool(name="outp", bufs=3))
        xsp = ctx.enter_context(tc.tile_pool(name="xsp", bufs=3))
        psum = ctx.enter_context(tc.tile_pool(name="psum", bufs=1, space="PSUM"))
        psum_xm = ctx.enter_context(tc.tile_pool(name="psum_xm", bufs=2, space="PSUM"))

        nc.gpsimd.load_library(library_config.mlp)

        ident = const.tile([P, P], F32)
        make_identity(nc, ident[:])
        ident_bf = const.tile([P, P], BF16)
        nc.scalar.copy(ident_bf[:], ident[:])
        wqT_s = const.tile([D, D], F32)
        nc.sync.dma_start(wqT_s[:], wqT)
        wk_s = const.tile([D, D], F32)
        nc.sync.dma_start(wk_s[:], wk)
        wvT_s = const.tile([D, D], F32)
        nc.sync.dma_start(wvT_s[:], wvT)
        bq_s = const.tile([D, 1], F32)
        nc.sync.dma_start(bq_s[:], bq)
        bv_s = const.tile([D, 1], F32)
        nc.sync.dma_start(bv_s[:], bv)
        idx_all = const.tile([P, idx_cols], I16)
        nc.sync.dma_start(idx_all[:], idxg)
        mask_all = const.tile([P, s_cols], F32)
        nc.sync.dma_start(mask_all[:], maskv)
        xsbf_all = const.tile([P, N_TILES * D], BF16)
        nc.sync.dma_start(xsbf_all[:], xsbf_d)

        for t in range(N_TILES):
            st = int(s_tiles[t])  # neighbor window slots
            ss = st + 1  # score slots incl self at col 0
            so = int(soff[t])
            xsT = xsp.tile([D, P], F32)
            nc.sync.dma_start(xsT[:], xsT_d[:, bass.ts(t, P)])
            xsbf = xsbf_all[:, bass.ts(t, D)]

            g = gpool.tile([P, st, D], F32)
            wo = 0
            for r in range(NR):
                mt = int(m[t, r])
                if mt == 0:
                    continue
                io = int(ioff[t, r])
                nc.gpsimd.dma_gather(
                    g[:, wo : wo + mt, :],
                    table[r * RANGE : (r + 1) * RANGE, :],
                    idx_all[:, io : io + mt * 8],
                    mt * P,
                    mt * P,
                    D,
                    single_packet=False,
                    queue_num=(t + r) % 4,
                )
                wo += mt

            # Q^T = Wq @ Xself^T + bq   [j, n]
            ps_qT = psum.tile([P, P], F32)
            nc.tensor.matmul(ps_qT[:], lhsT=wqT_s[:], rhs=xsT[:], start=True, stop=True)
            qT = small.tile([P, P], F32)
            nc.scalar.activation(
                qT[:], ps_qT[:],
                func=mybir.ActivationFunctionType.Identity,
                bias=bq_s[:, :1],
            )

            # Q' = Q @ Wk   [n, d]
            ps_qp = psum.tile([P, P], F32)
            nc.tensor.matmul(ps_qp[:], lhsT=qT[:], rhs=wk_s[:], start=True, stop=True)
            qp = small.tile([P, P], F32)
            nc.scalar.copy(qp[:], ps_qp[:])
            qp_bf = small.tile([P, P], BF16)
            nc.scalar.copy(qp_bf[:], ps_qp[:])

            # scores: col 0 self, cols 1..ss neighbors
            sc = small.tile([P, ss], F32)
            prods = small.tile([P, D], F32)
            nc.vector.tensor_tensor(
                prods[:], xsbf, qp_bf[:], op=mybir.AluOpType.mult
            )
            nc.vector.tensor_reduce(
                sc[:, 0:1], prods[:], axis=mybir.AxisListType.X,
                op=mybir.AluOpType.add,
            )
            prod = prodp.tile([P, st, D], F32)
            nc.vector.tensor_tensor(
                prod[:], g[:],
                qp[:, None, :].to_broadcast([P, st, D]),
                op=mybir.AluOpType.mult,
            )
            nc.vector.tensor_reduce(
                sc[:, 1:ss], prod[:], axis=mybir.AxisListType.X,
                op=mybir.AluOpType.add,
            )
            scm = small.tile([P, ss], F32)
            nc.vector.tensor_tensor(
                scm[:], sc[:], mask_all[:, so : so + ss], op=mybir.AluOpType.add
            )

            # softmax over score slots
            negmax = small.tile([P, 1], F32)
            nc.vector.tensor_reduce(
                negmax[:], scm[:], axis=mybir.AxisListType.X,
                op=mybir.AluOpType.max, negate=True,
            )
            e = small.tile([P, ss], F32)
            zsum = small.tile([P, 1], F32)
            nc.scalar.activation(
                e[:], scm[:],
                func=mybir.ActivationFunctionType.Exp,
                bias=negmax[:, :1],
                accum_out=zsum[:],
            )
            zinv = small.tile([P, 1], F32)
            nc.vector.reciprocal(zinv[:], zsum[:])
            attn = small.tile([P, ss], BF16)
            nc.vector.tensor_scalar_mul(attn[:], e[:], zinv[:, :1])

            # diag_s = diag(attn[:, s])  [P, ss, D]
            diag = diagp.tile([P, ss, D], BF16)
            nc.vector.tensor_tensor(
                diag[:],
                ident_bf[:, None, :].to_broadcast([P, ss, D]),
                attn[:, :, None].to_broadcast([P, ss, D]),
                op=mybir.AluOpType.mult,
            )

            gbf = gbfp.tile([P, st, D], BF16)
            nc.scalar.copy(gbf[:], g[:])

            # Xmix^T = Xself^T@diag_0 + sum_s G_s^T @ diag_{s+1}
            ps_xm = psum_xm.tile([P, P], F32)
            nc.tensor.matmul(
                ps_xm[:], lhsT=xsbf, rhs=diag[:, 0, :], start=True, stop=False
            )
            for s in range(st):
                nc.tensor.matmul(
                    ps_xm[:], lhsT=gbf[:, s, :], rhs=diag[:, s + 1, :],
                    start=False, stop=(s == st - 1),
                )
            xmT = small.tile([P, P], F32)
            nc.scalar.copy(xmT[:], ps_xm[:])

            # out^T = Wv @ Xmix^T + bv   [j, n]
            ps_mx = psum.tile([P, P], F32)
            nc.tensor.matmul(ps_mx[:], lhsT=wvT_s[:], rhs=xmT[:], start=True, stop=True)
            o_t = outp.tile([P, P], F32)
            nc.scalar.activation(
                o_t[:], ps_mx[:],
                func=mybir.ActivationFunctionType.Identity,
                bias=bv_s[:, :1],
            )
            nc.sync.dma_start(out[:, bass.ts(t, P)], o_t[:])

    nc.compile()
    return nc


_CACHE = {}


def prepare(inputs):
    table = np.ascontiguousarray(np.asarray(inputs["table"], dtype=np.float32))
    node = np.asarray(inputs["node"]).astype(np.int64)
    neighs = np.asarray(inputs["neighs"]).astype(np.int64)
    Wq = np.asarray(inputs["Wq"], dtype=np.float32)
    bq = np.asarray(inputs["bq"], dtype=np.float32)
    Wk = np.asarray(inputs["Wk"], dtype=np.float32)
    Wv = np.asarray(inputs["Wv"], dtype=np.float32)
    bv = np.asarray(inputs["bv"], dtype=np.float32)

    common = {
        "table": table,
        "wqT": np.ascontiguousarray(Wq.T),
        "wk": np.ascontiguousarray(Wk),
        "wvT": np.ascontiguousarray(Wv.T),
        "bq": np.ascontiguousarray(bq[:, None]),
        "bv": np.ascontiguousarray(bv[:, None]),
    }

    perms, m_all, nb_g, rng_g = plan_global(neighs)
    node_g = np.zeros(N_GLOBAL_PAD, dtype=np.int64)
    node_g[:N_NODES] = node
    s_tiles = m_all.sum(axis=1)
    ioff = np.zeros((N_TILES, NR), dtype=np.int64)
    soff = np.zeros(N_TILES, dtype=np.int64)
    io = 0
    so = 0
    for t in range(N_TILES):
        soff[t] = so
        for r in range(NR):
            ioff[t, r] = io
            io += int(m_all[t, r]) * 8
        so += int(s_tiles[t]) + 1  # +1 self col
    idx_cols, s_cols = io, so

    in_maps = []
    for c in range(N_CORES):
        idxg, maskv = pack_core(
            perms[c], nb_g, rng_g, m_all, ioff, soff, idx_cols, s_cols
        )
        nd_sorted = node_g[perms[c]]
        xs = table[nd_sorted]  # [N_PAD, D] f32 (pad nodes -> row of node 0: fine)
        xsT = np.ascontiguousarray(xs.T)  # [D, N_PAD]
        xsbf = np.ascontiguousarray(
            xs.reshape(N_TILES, P, D).transpose(1, 0, 2).reshape(P, N_TILES * D)
            .astype(ml_dtypes.bfloat16)
        )
        in_maps.append(dict(
            common,
            idxg=np.ascontiguousarray(idxg),
            maskv=np.ascontiguousarray(maskv),
            xsT=xsT,
            xsbf=xsbf,
        ))

    key = (idx_cols, s_cols, m_all.tobytes())
    if key not in _CACHE:
        _CACHE[key] = build_kernel(m_all, ioff, soff, s_tiles, idx_cols, s_cols)
    nc = _CACHE[key]
    return nc, in_maps, perms


def kernel(**inputs) -> np.ndarray:
    nc, in_maps, perms = prepare(inputs)
    results = run_bass_kernel_spmd(nc, in_maps, list(range(N_CORES))).results

    out = np.empty((N_NODES, D), dtype=np.float32)
    for c in range(N_CORES):
        o = results[c]["out"].T  # [N_PAD, D] rows in sorted order
        gid = perms[c]
        valid = gid < N_NODES
        out[gid[valid]] = o[valid]
    return out


if __name__ == "__main__":
    rng = np.random.default_rng(0)
    inputs = {
        "table": rng.standard_normal((VOCAB, D), dtype=np.float32),
        "node": rng.integers(0, VOCAB, (N_NODES,)),
        "neighs": rng.integers(0, VOCAB, (N_NODES, S)),
        "Wq": rng.uniform(-0.09, 0.09, (D, D)).astype(np.float32),
        "bq": rng.uniform(-0.09, 0.09, (D,)).astype(np.float32),
        "Wk": rng.uniform(-0.09, 0.09, (D, D)).astype(np.float32),
        "bk": rng.uniform(-0.09, 0.09, (D,)).astype(np.float32),
        "Wv": rng.uniform(-0.09, 0.09, (D, D)).astype(np.float32),
        "bv": rng.uniform(-0.09, 0.09, (D,)).astype(np.float32),
    }
    res = kernel(**inputs)
    print("kernel ran, output shape", res.shape)
